# revision 29
# baseline (speedup 1.0000x reference)
"""MoE (top-2 of 8 experts + shared expert) Trainium2 kernel, expert-parallel
across 8 NeuronCores, hybrid fp16/fp8 precision.  ~341-347us vs the 445us
bf16 baseline (~1.29x), rel err ~1.70e-2 (gate 2e-2), sim-exact.

Why it works: a routed expert-visit's error contribution to the final output
is attenuated by its gate combine weight (mean ~0.24, weight-1 shared expert
dominates the output norm).  So precision is allocated by weight:
  - The lowest-weight visits carrying FRAC=0.54 of routed sum-w^2 (~3/4 of
    visits) run fully in fp8 e4m3 with DoubleRow matmuls: 256-deep
    contraction per pass, 2x PE rate (measured 1.92x).
  - The high-weight rest + the shared expert run in fp16 (same PE rate as
    bf16, 4 more mantissa bits).  Their l-channel (linear-path) w1/w3 blocks
    are still fp8 -- mixed fp8-stationary x fp16-moving matmuls run at full
    rate and the l-path error sensitivity is damped (shared keeps fp16
    l-weights; its norm dominates).
  - Quantization scales (w x32, x x8 for fp8, h x2) keep values out of
    denormals and fold into ACT scale/bias constants: zero extra device ops
    (5 DVE + 3 ACT per i-tile, same as the bf16 baseline).

Planning (host, per call): float64 gate -> per-expert visit lists sorted by
weight -> cost-ordered search over slot structures {fp8 caps f1/f2 | p16 cap
c1...} where slot cost = max(PE-units, weight-DMA floor @ ~250GB/s) +
per-slot overhead; a vectorized DP assigns 8 instances per cap to experts
minimizing fp8 sum-w^2 under the budget; tokens split bottom-k fp8 / rest
p16.  fp8 caps are kept >= 352 so per-i-tile PE time stays above the
~2.0us/tile weight-DMA cadence (thin slots stall mid-GEMM1).  Typical plan:
per core [fp8 464 + fp8 352 + shared 512 + p16 256].

Device schedule: fp8 slots first (small first weight tiles -> fast fill),
shared second, thin p16 routed slot last (prefetched during shared's long
GEMM2 window).  All weight streams on the in-order Sync queue (measured:
striping across queues or moving w2 to gpsimd LOSES; the gpsimd bulk path is
slow); x/bias on GpSimd, y writebacks on Scalar.  All 8 w2 tiles prefetch
during GEMM1; 8 wf tiles of the next slot prefetch during GEMM2; first 6 wf
tiles of each slot split per-wi chunks for finer arrival.  Weight pools are
split g/l so fp8 l-tiles cost half the SBUF.

Beware when iterating: the device clock throttles run-to-run (observed
2.32 -> 1.94GHz); A/B-compare schedule variants back-to-back in one process
(bench_variants.py), never across separate runs.
"""
import sys

sys.path.insert(0, "/opt/trn_rl_repo")

import itertools
import os

import ml_dtypes
import numpy as np

import concourse.bacc as bacc_mod
import concourse.tile as tile
from concourse import mybir
from concourse.bass_utils import run_bass_kernel_spmd

F32 = mybir.dt.float32
FP16 = mybir.dt.float16
FP8 = mybir.dt.float8e4
NP_F8 = ml_dtypes.float8_e4m3
Alu = mybir.AluOpType
Act = mybir.ActivationFunctionType

ALPHA = 1.702
LIMIT = 7.0
TOPK = 2
D, I, E = 1024, 2048, 8
B, S = 2, 2048
T = B * S
DK = D // 128          # 8 d-tiles
IT = I // 128          # 16 i-tiles
TS = 512               # shared-expert tokens per core (T / 8)
N_CORES = 8
NB = 4 * IT + DK       # bias-pack columns

# fraction of routed sum-w^2 allowed into fp8 (error budget)
FRAC = float(os.environ.get("MOE_FRAC", "0.54"))
FP8_COST = 0.52        # measured fp8 PE cost per token vs fp16
DMA16 = 243            # p16 routed slot floor (16.8MB @ ~250GB/s, fp8 l-weights)
DMA8 = 151             # fp8 slot weight-stream floor (10.5MB @ ~250GB/s)
SLOT_OH = 16           # per-slot pipeline overhead, token units

# per-class constants: s1 = SX*SW is the GEMM1 psum scale
CLS = {
    "p16": dict(dt=FP16, npdt=np.float16, SX=1.0, SW=32.0, SH=1.0, SW2=32.0,
                dbl=False),
    "p8": dict(dt=FP8, npdt=NP_F8, SX=8.0, SW=32.0, SH=2.0, SW2=32.0,
               dbl=True),
}

# scheduling variant knobs (A/B benchable; cache key includes them)
SCHED = dict(w2q="sync", stripe=False, split6=True, pref=10)

_kernel_cache = {}


# --------------------------------------------------------------------------
# host-side packing
# --------------------------------------------------------------------------

def _q(a, cls):
    if cls == "p8":
        return np.clip(a, -240.0, 240.0).astype(NP_F8)
    return a.astype(np.float16)


def _tile13(w):
    """[D, I] -> [IT, 128(k), DK, 128(m)]."""
    return w.reshape(DK, 128, IT, 128).transpose(2, 1, 0, 3)


def _expert_pack(w1, b1, w3, b3, w2, b2, cls, l8=True):
    c = CLS[cls]
    s1 = c["SX"] * c["SW"]
    wfg = np.stack([_tile13(w1[:, 0::2]), _tile13(w3[:, 0::2])], axis=2)
    wfg = np.ascontiguousarray(wfg.reshape(IT, 128, 2 * DK, 128)) * c["SW"]
    wfl = np.stack([_tile13(w1[:, 1::2]), _tile13(w3[:, 1::2])], axis=2)
    wfl = np.ascontiguousarray(wfl.reshape(IT, 128, 2 * DK, 128)) * c["SW"]
    w2t = (w2 * (c["SW2"] / ALPHA)).reshape(IT, 128, DK, 128)
    w2t = np.ascontiguousarray(w2t.transpose(2, 1, 0, 3))  # [DK,128,IT,128]
    bias = np.concatenate([
        s1 * b1[0::2].reshape(IT, 128).T,
        b3[0::2].reshape(IT, 128).T,
        s1 * b1[1::2].reshape(IT, 128).T,
        (c["SH"] / s1) * b3[1::2].reshape(IT, 128).T,
        b2.reshape(DK, 128).T,
    ], axis=1)
    return {
        "wfg": _q(wfg, cls),
        "wfl": _q(wfl, "p8" if l8 else cls),
        "w2": _q(w2t, cls),
        "bias": np.ascontiguousarray(bias, dtype=np.float32),
    }


def _xt_pack(xsub, cap, cls):
    """[n, D] tokens -> zero-padded [128, DK, cap] transposed layout."""
    c = CLS[cls]
    n = xsub.shape[0]
    xt = np.zeros((D, cap), dtype=np.float32)
    xt[:, :n] = (c["SX"] * xsub).T
    xt = np.ascontiguousarray(xt.reshape(DK, 128, cap).transpose(1, 0, 2))
    return _q(xt, cls)


# --------------------------------------------------------------------------
# planning
# --------------------------------------------------------------------------

def _slot_cost(cap, cls):
    if cls == "p8":
        return max(FP8_COST * cap, DMA8) + SLOT_OH
    return max(cap, DMA16) + SLOT_OH


def _expert_combos(positions, ne, cume):
    """Instance-count combos (one tuple per position) covering ne tokens,
    p16-first fill; value = fp8 sum-w^2."""
    i16 = [i for i, (c, k) in enumerate(positions) if k == "p16"]
    i8s = [i for i, (c, k) in enumerate(positions) if k == "p8"]
    out = []
    seen = set()
    ranges = [range(9)] * len(i16)
    for c16 in itertools.product(*ranges):
        pcap = sum(n * positions[i][0] for n, i in zip(c16, i16))
        f8 = max(0, ne - pcap)
        covers = []
        if f8 == 0:
            covers.append([0] * len(i8s))
        elif len(i8s) == 1:
            j = -(-f8 // positions[i8s[0]][0])
            if j <= 8:
                covers.append([j])
        elif len(i8s) == 2:
            ca, cb = positions[i8s[0]][0], positions[i8s[1]][0]
            for j1 in range(9):
                j2 = -(-max(0, f8 - j1 * ca) // cb)
                if j2 <= 8:
                    covers.append([j1, j2])
                    if j2 == 0:
                        break
        for cov in covers:
            ix = [0] * len(positions)
            for n, i in zip(c16, i16):
                ix[i] = n
            for n, i in zip(cov, i8s):
                ix[i] = n
            ixt = tuple(ix)
            if ixt in seen:
                continue
            seen.add(ixt)
            out.append((ixt, float(cume[f8])))
    return out


def _assign_np(positions, n_e, cum, budget):
    """Vectorized DP over instance-usage states; returns per-expert combo
    tuples or None."""
    P = len(positions)
    shape = (9,) * P
    INF = np.inf
    val = np.full(shape, INF)
    val[(0,) * P] = 0.0
    order = sorted(range(E), key=lambda e: -n_e[e])
    trace = []
    for e in order:
        combos = _expert_combos(positions, n_e[e], cum[e])
        if not combos:
            return None
        nv = np.full(shape, INF)
        pidx = np.full(shape, -1, dtype=np.int32)
        for ci, (ix, w2v) in enumerate(combos):
            src = val[tuple(slice(0, 9 - i) for i in ix)] + w2v
            dst = nv[tuple(slice(i, 9) for i in ix)]
            pv = pidx[tuple(slice(i, 9) for i in ix)]
            m = src < dst
            dst[m] = src[m]
            pv[m] = ci
        nv[nv > budget] = INF
        if not np.isfinite(nv).any():
            return None
        trace.append((e, combos, pidx))
        val = nv
    state = np.unravel_index(np.argmin(val), shape)
    if not np.isfinite(val[state]):
        return None
    out = [None] * E
    for e, combos, pidx in reversed(trace):
        ci = int(pidx[state])
        ix = combos[ci][0]
        out[e] = ix
        state = tuple(s - i for s, i in zip(state, ix))
    return out


def _plan_hybrid(n_e, wt_sorted):
    """Cost-ordered search over slot structures with DP budget feasibility.
    Returns (positions, asg): positions = [(cap, cls)...], asg[e] = instance
    counts per position."""
    cum = [np.concatenate([[0.0], np.cumsum(w.astype(np.float64) ** 2)])
           for w in wt_sorted]
    budget = FRAC * sum(float(c[-1]) for c in cum)
    tot = sum(n_e)

    # fp8 caps >= 352 keep per-i-tile PE time above the ~2.0us wf-DMA
    # cadence (0.5MB/it at ~250GB/s) with margin -- thin slots stall
    f_grid = list(range(352, 897, 16))
    f2_grid = [0] + list(range(352, 897, 16))
    c_grid = list(range(224, 513, 16))
    cands = {}

    def add(poss):
        poss = tuple(sorted((p for p in poss if p[0] > 0),
                            key=lambda p: (p[1], -p[0])))
        if not poss or poss in cands:
            return
        if 8 * sum(c for c, _ in poss) < tot:
            return
        cands[poss] = sum(_slot_cost(c, k) for c, k in poss)

    for f1 in f_grid:
        for c1 in c_grid:
            add([(f1, "p8"), (c1, "p16")])
            for f2 in f2_grid:
                if f2 <= f1:
                    add([(f1, "p8"), (f2, "p8"), (c1, "p16")])
            for c2 in range(224, c1 + 1, 16):
                add([(f1, "p8"), (c1, "p16"), (c2, "p16")])
    for c1 in range(224, 513, 32):
        for c2 in range(224, c1 + 1, 32):
            add([(c1, "p16"), (c2, "p16")])
            for c3 in range(224, c2 + 1, 32):
                add([(c1, "p16"), (c2, "p16"), (c3, "p16")])

    for poss in sorted(cands, key=lambda p: cands[p]):
        asg = _assign_np(list(poss), n_e, cum, budget)
        if asg is not None:
            return list(poss), asg
    return None


# --------------------------------------------------------------------------
# device kernel
# --------------------------------------------------------------------------

def _groups(cap):
    gs = [512] * (cap // 512)
    if cap % 512:
        gs.append(cap % 512)
    offs = np.cumsum([0] + gs)[:-1]
    return list(zip(offs, gs))


def _build(slot_desc):
    """slot_desc: tuple of (cap, cls) in device order."""
    nc = bacc_mod.Bacc("TRN2")

    def dram(name, shape, dtype, out=False):
        return nc.declare_dram_parameter(name, list(shape), dtype, isOutput=out)

    slots = []
    for j, (cap, cls, lw8) in enumerate(slot_desc):
        p = f"s{j}"
        dt = CLS[cls]["dt"]
        w = {
            "xt": dram(p + "xt", [128, DK, cap], dt),
            "wfg": dram(p + "wfg", [IT, 128, 2 * DK, 128], dt),
            "wfl": dram(p + "wfl", [IT, 128, 2 * DK, 128],
                        FP8 if lw8 else dt),
            "w2": dram(p + "w2", [DK, 128, IT, 128], dt),
            "bias": dram(p + "bias", [128, NB], F32),
            "y": dram(p + "y", [DK, 128, cap], FP16, out=True),
        }
        slots.append((j, cap, cls, lw8, w))

    with tile.TileContext(nc) as tc:
        with (
            tc.tile_pool(name="persist", bufs=1) as persist,
            tc.tile_pool(name="wgpool", bufs=11) as wgpool,
            tc.tile_pool(name="wlpool", bufs=11) as wlpool,
            tc.tile_pool(name="w2pool", bufs=8) as w2pool,
            tc.tile_pool(name="work", bufs=2) as work,
            tc.tile_pool(name="outp", bufs=3) as outp,
            tc.tile_pool(name="ps", bufs=1, space="PSUM") as ps,
            tc.tile_pool(name="psy", bufs=3, space="PSUM") as psy,
        ):
            xts_t, bt_t, hb_t = {}, {}, {}
            for j, cap, cls, lw8, w in slots:
                dt = CLS[cls]["dt"]
                xts_t[j] = persist.tile([128, DK, cap], dt, tag=f"xt{j}",
                                        name=f"xt_s{j}")
                bt_t[j] = persist.tile([128, NB], F32, tag=f"bias{j}",
                                       name=f"bias_s{j}")
                hb_t[j] = persist.tile([128, IT, cap], dt, tag=f"h{j}",
                                       name=f"h_s{j}")

            def load_xt_bias(j):
                _, cap, _, _, w = slots[j]
                if j == 0:
                    xap = w["xt"].ap()
                    for q in range(0, DK, 2):
                        eng = nc.scalar if q < 4 else nc.gpsimd
                        eng.dma_start(out=xts_t[j][:, q:q + 2],
                                      in_=xap[:, q:q + 2])
                else:
                    nc.gpsimd.dma_start(out=xts_t[j], in_=w["xt"].ap())
                nc.gpsimd.dma_start(out=bt_t[j], in_=w["bias"].ap())

            load_xt_bias(0)

            def wf_load(j, it):
                _, _, cls, lw8, w = slots[j]
                dt = CLS[cls]["dt"]
                wtg = wgpool.tile([128, 2 * DK, 128], dt, tag="wfg",
                                  name=f"wfg_s{j}_{it}")
                wtl = wlpool.tile([128, 2 * DK, 128],
                                  FP8 if lw8 else dt, tag="wfl",
                                  name=f"wfl_s{j}_{it}")
                if j == 0 and it < 2:
                    # startup race: the Scalar queue may come up before Sync
                    for wi in range(2):
                        nc.scalar.dma_start(
                            out=wtg[:, wi * DK:(wi + 1) * DK, :],
                            in_=w["wfg"][it][:, wi * DK:(wi + 1) * DK, :])
                        nc.scalar.dma_start(
                            out=wtl[:, wi * DK:(wi + 1) * DK, :],
                            in_=w["wfl"][it][:, wi * DK:(wi + 1) * DK, :])
                    return wtg, wtl
                if it < 6:
                    # split per-wi: finer arrival during fills/transitions
                    for wi in range(2):
                        nc.sync.dma_start(
                            out=wtg[:, wi * DK:(wi + 1) * DK, :],
                            in_=w["wfg"][it][:, wi * DK:(wi + 1) * DK, :])
                        nc.sync.dma_start(
                            out=wtl[:, wi * DK:(wi + 1) * DK, :],
                            in_=w["wfl"][it][:, wi * DK:(wi + 1) * DK, :])
                else:
                    nc.sync.dma_start(out=wtg, in_=w["wfg"][it])
                    nc.sync.dma_start(out=wtl, in_=w["wfl"][it])
                return wtg, wtl

            wf_pre = {}
            for j, cap, cls, lw8, w in slots:
                c = CLS[cls]
                s1 = c["SX"] * c["SW"]
                dbl = c["dbl"]
                grp = _groups(cap)
                xts, bt, hb = xts_t[j], bt_t[j], hb_t[j]
                w2_pre = {}

                def prefetch_w2(dk, j=j, cls=cls, w=w, w2_pre=w2_pre):
                    w2t = w2pool.tile([128, IT, 128], CLS[cls]["dt"], tag="w2",
                                      name=f"w2_s{j}_{dk}")
                    if SCHED["w2q"] == "gpsimd" or (
                            SCHED["w2q"] == "stripe" and dk % 2):
                        nc.gpsimd.dma_start(out=w2t, in_=w["w2"][dk])
                    else:
                        nc.sync.dma_start(out=w2t, in_=w["w2"][dk])
                    w2_pre[dk] = w2t

                # ---- first GEMM + swiglu: h[it, tok] ----
                for it in range(IT):
                    wts = wf_pre.pop((j, it), None)
                    if wts is None:
                        wts = wf_load(j, it)
                    wtg, wtl = wts
                    if it == 8 and j + 1 < len(slots):
                        load_xt_bias(j + 1)
                    # all 8 w2 tiles land during GEMM1 so GEMM2 issues no
                    # Sync-queue loads behind the next slot's wf prefetch
                    W2_AT = {5: 0, 6: 1, 7: 2, 8: 3, 9: 4, 11: 5, 13: 6, 15: 7}
                    if it in W2_AT:
                        prefetch_w2(W2_AT[it])
                    for goff, gsz in grp:
                        accs = []
                        for wi in range(4):
                            acc = ps.tile([128, 512], F32, tag=f"acc{wi}",
                                          name=f"acc{wi}_s{j}_{it}_{goff}")
                            wt = wtg if wi < 2 else wtl
                            wb = (wi % 2) * DK
                            if dbl:
                                for p2 in range(DK // 2):
                                    nc.tensor.matmul(
                                        acc[:, :gsz],
                                        wt[:, wb + 2 * p2:wb + 2 * p2 + 2, :],
                                        xts[:, 2 * p2:2 * p2 + 2,
                                            goff:goff + gsz],
                                        start=(p2 == 0),
                                        stop=(p2 == DK // 2 - 1),
                                        perf_mode=mybir.MatmulPerfMode.DoubleRow)
                            else:
                                for dk in range(DK):
                                    nc.tensor.matmul(
                                        acc[:, :gsz],
                                        wt[:, wb + dk, :],
                                        xts[:, dk, goff:goff + gsz],
                                        start=(dk == 0), stop=(dk == DK - 1))
                            accs.append(acc)
                        A, Bm, C, Dm = accs
                        # Bp = v_e = B/s1 + b3e
                        Bp = work.tile([128, 512], F32, tag="Bp")
                        nc.scalar.activation(Bp[:, :gsz], Bm[:, :gsz],
                                             Act.Identity, scale=1.0 / s1,
                                             bias=bt[:, IT + it:IT + it + 1])
                        # G = (A + s1*b1e) * Bp = s1*g
                        G = work.tile([128, 512], F32, tag="G")
                        nc.vector.scalar_tensor_tensor(
                            G[:, :gsz], A[:, :gsz], bt[:, it:it + 1],
                            Bp[:, :gsz], Alu.add, Alu.mult)
                        nc.vector.tensor_scalar_min(G[:, :gsz], G[:, :gsz],
                                                    LIMIT * s1)
                        # Sv = Silu(alpha*g) = alpha*g*sig(alpha*g)
                        Sv = work.tile([128, 512], F32, tag="Sv")
                        nc.scalar.activation(Sv[:, :gsz], G[:, :gsz],
                                             Act.Silu, scale=ALPHA / s1)
                        # Dp = (SH/s1)*v_o
                        Dp = work.tile([128, 512], F32, tag="Dp")
                        nc.scalar.activation(
                            Dp[:, :gsz], Dm[:, :gsz], Act.Identity,
                            scale=c["SH"] / (s1 * s1),
                            bias=bt[:, 3 * IT + it:3 * IT + it + 1])
                        # L = (C + s1*b1o) * Dp = SH*l
                        L = work.tile([128, 512], F32, tag="L")
                        nc.vector.scalar_tensor_tensor(
                            L[:, :gsz], C[:, :gsz],
                            bt[:, 2 * IT + it:2 * IT + it + 1],
                            Dp[:, :gsz], Alu.add, Alu.mult)
                        nc.vector.tensor_scalar(L[:, :gsz], L[:, :gsz],
                                                LIMIT * c["SH"],
                                                -LIMIT * c["SH"],
                                                Alu.min, Alu.max)
                        # hb = (L + SH)*Sv = SH*alpha*h
                        nc.vector.scalar_tensor_tensor(
                            hb[:, it, goff:goff + gsz],
                            L[:, :gsz], c["SH"], Sv[:, :gsz],
                            Alu.add, Alu.mult)

                if j + 1 < len(slots):
                    for it2 in range(SCHED["pref"]):
                        wf_pre[(j + 1, it2)] = wf_load(j + 1, it2)

                # ---- second GEMM: y[dk] = sum_it w2[dk,it].T @ h[it] ----
                y_scale = 1.0 / (c["SH"] * c["SW2"])
                for dk in range(DK):
                    if dk in w2_pre:
                        w2t = w2_pre.pop(dk)
                    else:
                        w2t = w2pool.tile([128, IT, 128], c["dt"], tag="w2",
                                          name=f"w2_s{j}_{dk}")
                        nc.sync.dma_start(out=w2t, in_=w["w2"][dk])
                    for goff, gsz in grp:
                        Y = psy.tile([128, 512], F32, tag="Y",
                                     name=f"Y_s{j}_{dk}_{goff}")
                        if dbl:
                            for p2 in range(IT // 2):
                                nc.tensor.matmul(
                                    Y[:, :gsz],
                                    w2t[:, 2 * p2:2 * p2 + 2, :],
                                    hb[:, 2 * p2:2 * p2 + 2, goff:goff + gsz],
                                    start=(p2 == 0), stop=(p2 == IT // 2 - 1),
                                    perf_mode=mybir.MatmulPerfMode.DoubleRow)
                        else:
                            for it in range(IT):
                                nc.tensor.matmul(
                                    Y[:, :gsz],
                                    w2t[:, it, :],
                                    hb[:, it, goff:goff + gsz],
                                    start=(it == 0), stop=(it == IT - 1))
                        yo = outp.tile([128, 512], FP16, tag="yo")
                        nc.scalar.activation(
                            yo[:, :gsz], Y[:, :gsz], Act.Identity,
                            scale=y_scale,
                            bias=bt[:, 4 * IT + dk:4 * IT + dk + 1])
                        nc.scalar.dma_start(
                            out=w["y"][dk, :, goff:goff + gsz],
                            in_=yo[:, :gsz])

    nc.finalize()
    return nc


# --------------------------------------------------------------------------
# host plan construction (shared with the accuracy sim)
# --------------------------------------------------------------------------

def _make_plan(xt, gate_w, gate_b):
    """Gate + routing + slot planning.  Returns a dict with everything the
    packer/combiner needs."""
    z = xt.astype(np.float64) @ np.asarray(gate_w, dtype=np.float64).T
    z -= z.max(axis=-1, keepdims=True)
    ez = np.exp(z)
    scores = ez / ez.sum(axis=-1, keepdims=True)
    biased = scores + np.asarray(gate_b, dtype=np.float64)
    top2 = np.argsort(-biased, axis=-1, kind="stable")[:, :TOPK]
    gate_wt = np.take_along_axis(scores, top2, axis=-1).astype(np.float32)

    tok_sorted, wt_sorted = [], []
    for e in range(E):
        sel = np.nonzero((top2 == e).any(axis=1))[0]
        we = np.where(top2[sel, 0] == e, gate_wt[sel, 0],
                      gate_wt[sel, 1]).astype(np.float32)
        order = np.argsort(we, kind="stable")
        tok_sorted.append(sel[order])
        wt_sorted.append(we[order])
    n_e = [len(s) for s in tok_sorted]

    plan = _plan_hybrid(n_e, wt_sorted)
    if plan is None:
        raise RuntimeError("no hybrid plan found")
    positions, asg = plan
    p8pos = sorted((i for i, (c, k) in enumerate(positions) if k == "p8"),
                   key=lambda i: -positions[i][0])
    p16pos = sorted((i for i, (c, k) in enumerate(positions) if k == "p16"),
                    key=lambda i: -positions[i][0])

    # token split per expert: bottom take8 go fp8, rest p16
    take8 = []
    for e in range(E):
        pc = sum(asg[e][i] * positions[i][0] for i in p16pos)
        take8.append(max(0, n_e[e] - pc))

    # pieces per position (larger caps filled first within each class)
    pieces = {i: [] for i in range(len(positions))}
    for e in range(E):
        lo = 0
        for i in p8pos:
            for _ in range(asg[e][i]):
                hi = min(lo + positions[i][0], take8[e])
                pieces[i].append((e, lo, hi))
                lo = hi
        assert lo >= take8[e]
        lo = take8[e]
        for i in p16pos:
            for _ in range(asg[e][i]):
                hi = min(lo + positions[i][0], n_e[e])
                pieces[i].append((e, lo, hi))
                lo = hi
        assert lo >= n_e[e]
    for i in pieces:
        assert len(pieces[i]) <= N_CORES
        while len(pieces[i]) < N_CORES:
            pieces[i].append((0, 0, 0))

    # device slot order: fp8 first (cheap fill); shared's long GEMM2 window
    # prefetches the thin p16 routed slot(s), which run last
    entries = [(("pos", i), positions[i][0], "p8") for i in p8pos]
    entries.append(("sh", TS, "p16"))
    entries += [(("pos", i), positions[i][0], "p16") for i in p16pos]

    return dict(top2=top2, gate_wt=gate_wt, tok_sorted=tok_sorted,
                wt_sorted=wt_sorted, n_e=n_e, positions=positions,
                take8=take8, pieces=pieces, entries=entries)


# --------------------------------------------------------------------------
# entry point
# --------------------------------------------------------------------------

def kernel(x, gate_w, gate_b, w1, b1, w3, b3, w2, b2,
           sw1, sb1, sw3, sb3, sw2, sb2):
    x = np.asarray(x, dtype=np.float32)
    xt = x.reshape(T, D)

    plan = _make_plan(xt, gate_w, gate_b)
    entries = plan["entries"]
    pieces = plan["pieces"]
    tok_sorted, wt_sorted = plan["tok_sorted"], plan["wt_sorted"]

    epacks = {}

    def epack(e, cls):
        if (e, cls) not in epacks:
            epacks[(e, cls)] = _expert_pack(
                np.asarray(w1[e]), np.asarray(b1[e]), np.asarray(w3[e]),
                np.asarray(b3[e]), np.asarray(w2[e]), np.asarray(b2[e]), cls,
                l8=True)
        return epacks[(e, cls)]

    spack = _expert_pack(np.asarray(sw1), np.asarray(sb1), np.asarray(sw3),
                         np.asarray(sb3), np.asarray(sw2), np.asarray(sb2),
                         "p16", l8=False)

    in_maps = []
    for cc in range(N_CORES):
        m = {}
        for s, (kind, cap, cls) in enumerate(entries):
            if kind == "sh":
                m[f"s{s}xt"] = _xt_pack(xt[cc * TS:(cc + 1) * TS], TS, cls)
                pk = spack
            else:
                e, lo, hi = pieces[kind[1]][cc]
                m[f"s{s}xt"] = _xt_pack(xt[tok_sorted[e][lo:hi]], cap, cls)
                pk = epack(e, cls)
            for kk, v in pk.items():
                m[f"s{s}{kk}"] = v
        in_maps.append(m)

    slot_desc = tuple((cap, cls, kind != "sh")
                      for kind, cap, cls in entries)
    ckey = (slot_desc, tuple(sorted(SCHED.items())))
    if ckey not in _kernel_cache:
        _kernel_cache[ckey] = _build(slot_desc)
    nc = _kernel_cache[ckey]
    res = run_bass_kernel_spmd(nc, in_maps, list(range(N_CORES)))

    # ---- combine ----
    out = np.zeros((T, D), dtype=np.float32)
    for cc in range(N_CORES):
        for s, (kind, cap, cls) in enumerate(entries):
            yc = res.results[cc][f"s{s}y"].astype(np.float32)
            yc = yc.reshape(D, cap)
            if kind == "sh":
                out[cc * TS:(cc + 1) * TS] += yc.T
            else:
                e, lo, hi = pieces[kind[1]][cc]
                if hi <= lo:
                    continue
                idx = tok_sorted[e][lo:hi]
                out[idx] += wt_sorted[e][lo:hi][:, None] * yc.T[:hi - lo]
    return out.reshape(B, S, D)
